# revision 3
# baseline (speedup 1.0000x reference)
import sys

for _p in ("/opt/trn_rl_repo", "/root/.axon_site/_ro/trn_rl_repo"):
    if _p not in sys.path:
        sys.path.insert(0, _p)

import numpy as np
import ml_dtypes

import concourse.bass as bass
import concourse.mybir as mybir
import concourse.tile as tile
from concourse import bacc
from concourse import bass_utils

BF16 = ml_dtypes.bfloat16

P = 128
B = 8
T = 1024
S0 = 1500
S = 1536
D = 1024
H = 16
Dh = 64
DT = D // P
ST = S // P
NPAIR = H // 2
HW = Dh + 1
SCALE = Dh ** -0.5

f32 = mybir.dt.float32
bf16 = mybir.dt.bfloat16


def build_bass():
    nc = bacc.Bacc("TRN2", target_bir_lowering=False, debug=False,
                   enable_asserts=False, num_devices=B)

    xT_d = nc.dram_tensor("xT", [D, T], bf16, kind="ExternalInput")
    kT_d = nc.dram_tensor("kT", [D, S], bf16, kind="ExternalInput")
    va_d = nc.dram_tensor("vaug", [S, H * HW], bf16, kind="ExternalInput")
    wqT_d = nc.dram_tensor("wqT", [D, D], bf16, kind="ExternalInput")
    bq_d = nc.dram_tensor("bqr", [P, DT], f32, kind="ExternalInput")
    woT_d = nc.dram_tensor("woT", [D, D], bf16, kind="ExternalInput")
    bo_d = nc.dram_tensor("bor", [P, DT], f32, kind="ExternalInput")
    outT_d = nc.dram_tensor("outT", [D, T], f32, kind="ExternalOutput")

    EXP = mybir.ActivationFunctionType.Exp

    with tile.TileContext(nc) as tc:
        with (
            tc.tile_pool(name="const", bufs=1) as cp,
            tc.tile_pool(name="work", bufs=2) as wp,
            tc.tile_pool(name="psum_sc", bufs=2, space="PSUM") as scp,
            tc.tile_pool(name="psum_pv", bufs=3, space="PSUM") as pvp,
            tc.tile_pool(name="psum_qp", bufs=1, space="PSUM") as qpp,
        ):
            def mk(cols, nm, dt=bf16):
                return cp.tile([P, cols], dt, name=nm, tag=nm)

            xTh_sb = {(j, th): mk(512, f"xT{j}_{th}")
                      for j in range(DT) for th in range(2)}
            wqT_sb = [mk(D, f"wqTs{j}") for j in range(DT)]
            kT_sb = [mk(S, f"kTs{j}") for j in range(DT)]
            va_sb = [mk(H * HW, f"vas{c}") for c in range(ST)]
            woT_sb = [mk(D, f"woTs{j}") for j in range(DT)]
            bq_sb = mk(DT, "bq_sb", f32)
            bo_sb = mk(DT, "bo_sb", f32)
            qT_sb = [mk(T, f"qTs{j}") for j in range(DT)]
            aT_sb = [mk(T, f"aTs{j}") for j in range(DT)]

            for dt_i in range(DT):
                e1, e2 = ((nc.sync, nc.scalar) if dt_i % 2 == 0
                          else (nc.scalar, nc.sync))
                e1.dma_start(wqT_sb[dt_i][:],
                             wqT_d[dt_i * P:(dt_i + 1) * P, :])
                e2.dma_start(xTh_sb[(dt_i, 0)][:],
                             xT_d[dt_i * P:(dt_i + 1) * P, 0:512])
            nc.scalar.dma_start(bq_sb[:], bq_d[:, :])
            nc.sync.dma_start(kT_sb[0][:], kT_d[0:P, :])
            for c in range(4):
                nc.sync.dma_start(va_sb[c][:], va_d[c * P:(c + 1) * P, :])
            for j in range(DT):
                nc.sync.dma_start(xTh_sb[(j, 1)][:],
                                  xT_d[j * P:(j + 1) * P, 512:1024])
            for j in range(1, DT):
                nc.sync.dma_start(kT_sb[j][:], kT_d[j * P:(j + 1) * P, :])
            for c in range(4, ST):
                nc.sync.dma_start(va_sb[c][:], va_d[c * P:(c + 1) * P, :])
            for j in range(DT):
                nc.sync.dma_start(woT_sb[j][:], woT_d[j * P:(j + 1) * P, :])
            nc.sync.dma_start(bo_sb[:], bo_d[:, :])

            def q_chain_ops(j, tch):
                tsl = slice(tch * 512, (tch + 1) * 512)
                ps = qpp.tile([P, 512], f32, name=f"qp{j}_{tch}", tag="qp")
                ops = []
                for dt_i in range(DT):
                    def mm(dt_i=dt_i, ps=ps, tch=tch, j=j):
                        nc.tensor.matmul(
                            ps[:, :],
                            lhsT=wqT_sb[dt_i][:, j * P:(j + 1) * P],
                            rhs=xTh_sb[(dt_i, tch)][:, :],
                            start=(dt_i == 0), stop=(dt_i == DT - 1),
                        )
                    ops.append(mm)

                def evict(ps=ps, tsl=tsl, j=j):
                    nc.vector.tensor_scalar_add(qT_sb[j][:, tsl], ps[:, :],
                                                bq_sb[:, j:j + 1])
                ops.append(evict)
                return ops

            for op in q_chain_ops(0, 0):
                op()

            fifo = []
            for j in range(1, DT):
                fifo.extend(q_chain_ops(j, 0))
            for j in range(DT):
                fifo.extend(q_chain_ops(j, 1))

            def out_chain_ops(fj, tch):
                tsl = slice(tch * 512, (tch + 1) * 512)
                ps = qpp.tile([P, 512], f32, name=f"op{fj}_{tch}", tag="qp")
                ops = []
                for et in range(DT):
                    def mm(et=et, ps=ps, fj=fj, tsl=tsl):
                        nc.tensor.matmul(
                            ps[:, :],
                            lhsT=woT_sb[et][:, fj * P:(fj + 1) * P],
                            rhs=aT_sb[et][:, tsl],
                            start=(et == 0), stop=(et == DT - 1),
                        )
                    ops.append(mm)

                def evict(ps=ps, fj=fj, tch=tch, tsl=tsl):
                    ost = wp.tile([P, 512], f32, name=f"ost{fj}_{tch}",
                                  tag="ost", bufs=4)
                    nc.vector.tensor_scalar_add(ost[:, :], ps[:, :],
                                                bo_sb[:, fj:fj + 1])
                    nc.sync.dma_start(
                        outT_d[fj * P:(fj + 1) * P, tsl], ost[:, :])
                ops.append(evict)
                return ops

            halves = [(j, th) for th in range(2) for j in range(NPAIR)]
            steps = [(h, c) for h in range(len(halves)) for c in range(ST)]
            NSTEP = len(steps)

            sc_t = [None] * NSTEP
            pt_t = [None] * NSTEP
            pv_t = {}

            def emit_sc(i):
                h, c = steps[i]
                j, th = halves[h]
                tsl = slice(th * 512, (th + 1) * 512)
                csl = slice(c * P, (c + 1) * P)
                sc = scp.tile([P, T], f32, name=f"sc{i}", tag="sc")
                sc_t[i] = sc
                for a in range(2):
                    rows = slice(a * Dh, (a + 1) * Dh)
                    nc.tensor.matmul(
                        sc[:, a * 512:(a + 1) * 512],
                        lhsT=kT_sb[j][rows, csl],
                        rhs=qT_sb[j][rows, tsl],
                        start=True, stop=True,
                    )

            def emit_exp(i):
                pt = wp.tile([P, T], bf16, name=f"pt{i}", tag="pt", bufs=4)
                pt_t[i] = pt
                nc.scalar.activation(pt[:, :], sc_t[i][:, :], EXP)

            def emit_pv(i):
                h, c = steps[i]
                j, th = halves[h]
                if c == 0:
                    pv_t[h] = [pvp.tile([HW, 512], f32, name=f"pv{h}_{a}",
                                        tag="pv") for a in range(2)]
                for a in range(2):
                    hh = 2 * j + a
                    nc.tensor.matmul(
                        pv_t[h][a][0:HW, :],
                        lhsT=va_sb[c][:, hh * HW:(hh + 1) * HW],
                        rhs=pt_t[i][:, a * 512:(a + 1) * 512],
                        start=(c == 0), stop=(c == ST - 1),
                    )

            def emit_norm(h):
                j, th = halves[h]
                tsl = slice(th * 512, (th + 1) * 512)
                for a in range(2):
                    pvsb = wp.tile([HW, 512], f32, name=f"pvsb{h}_{a}",
                                   tag="pvsb", bufs=4)
                    nc.vector.tensor_copy(pvsb[:, :], pv_t[h][a][0:HW, :])
                    dsm = wp.tile([Dh, 8], f32, name=f"ds{h}_{a}",
                                  tag="dsm", bufs=4)
                    nc.sync.dma_start(dsm[:, :], pvsb[Dh:Dh + 1, :])
                    nc.vector.reciprocal(dsm[:, :], dsm[:, :])
                    rrow = wp.tile([1, 512], f32, name=f"rr{h}_{a}",
                                   tag="rrow", bufs=4)
                    nc.sync.dma_start(rrow[:, :], dsm[:, :])
                    nrm = wp.tile([Dh, 512], f32, name=f"nr{h}_{a}",
                                  tag="nrm", bufs=4)
                    nc.gpsimd.partition_broadcast(nrm[:, :], rrow[0:1, :])
                    nc.vector.tensor_mul(
                        aT_sb[j][a * Dh:(a + 1) * Dh, tsl],
                        pvsb[0:Dh, :], nrm[:, :])

            emit_sc(0)
            for i in range(NSTEP):
                emit_exp(i)
                if i + 1 < NSTEP:
                    emit_sc(i + 1)
                if i >= 1:
                    emit_pv(i - 1)
                    ph, pc = steps[i - 1]
                    if pc == ST - 1:
                        emit_norm(ph)
                        if ph == NPAIR - 1:
                            for fj in range(DT):
                                fifo.extend(out_chain_ops(fj, 0))
                if fifo:
                    fifo.pop(0)()
            emit_pv(NSTEP - 1)
            emit_norm(len(halves) - 1)

            for op in fifo:
                op()
            for fj in range(DT):
                for op in out_chain_ops(fj, 1):
                    op()

    nc.compile()
    return nc


def prep_inputs(x, k, v, wq, bq, wo, bo):
    x = np.asarray(x, np.float32)
    k = np.asarray(k, np.float32)
    v = np.asarray(v, np.float32)
    wq = np.asarray(wq, np.float32)
    bq = np.asarray(bq, np.float32)
    wo = np.asarray(wo, np.float32)
    bo = np.asarray(bo, np.float32)

    wqT = np.ascontiguousarray((wq * SCALE).T).astype(BF16)
    bqr = np.ascontiguousarray((bq * SCALE).reshape(DT, P).T)
    woT = np.ascontiguousarray(wo.T).astype(BF16)
    bor = np.ascontiguousarray(bo.reshape(DT, P).T)

    in_maps = []
    for b in range(x.shape[0]):
        xT = np.ascontiguousarray(x[b].T).astype(BF16)
        kT = np.zeros((D, S), BF16)
        kT[:, :S0] = k[b].T.astype(BF16)
        vaug = np.zeros((S, H * HW), BF16)
        vb = v[b].astype(BF16)
        for h in range(H):
            vaug[:S0, h * HW:h * HW + Dh] = vb[:, h * Dh:(h + 1) * Dh]
            vaug[:S0, h * HW + Dh] = BF16(1.0)
        in_maps.append({
            "xT": xT, "kT": kT, "vaug": np.ascontiguousarray(vaug),
            "wqT": wqT, "bqr": bqr, "woT": woT, "bor": bor,
        })
    return in_maps


_NC_CACHE = {}


def kernel(x, k, v, wq, bq, wo, bo, _trace=False):
    if "nc" not in _NC_CACHE:
        _NC_CACHE["nc"] = build_bass()
    nc = _NC_CACHE["nc"]
    in_maps = prep_inputs(x, k, v, wq, bq, wo, bo)
    res = bass_utils.run_bass_kernel_spmd(
        nc, in_maps, core_ids=list(range(B)), trace=_trace)
    _NC_CACHE["last_result"] = res
    out = np.stack([np.ascontiguousarray(r["outT"].T) for r in res.results])
    return out
